# revision 2
# baseline (speedup 1.0000x reference)
"""Causal self-attention TRN2 Bass kernel v4 (B=4, T=2048, C=1024, H=16, D=64).

Sharding: 8 cores = 4 batches x 2 head-groups (8 heads each); host sums the two
head-group partial projections per batch and adds b_proj.

All matmuls bf16 (tolerance 2e-2); q/k/v SBUF-resident; v written directly in
the ones-augmented AV layout.

Scheduling: softmax exp on ACT (853ns/step) is slower than the S+AV PE work
per attention step (854ns incl. overheads ACT ~1040ns), so attention alone is
ACT-bound. v4 makes PE work-conserving: QKV for token chunk tch+1 and the
projection for chunk tch-1 are emitted as paced filler between attention
AV-steps of chunk tch (causality: query chunk tch needs K/V only through
chunk tch). ACT does exp ONLY; all bias/copy epilogues run on DVE.
PSUM: 2 S-tiles (4 banks) + 2 O-tiles (2) + 1 filler accumulator (2) = 8.
"""

import numpy as np
from contextlib import ExitStack

import concourse.bass as bass
import concourse.tile as tile
from concourse import bacc, mybir
from concourse.bass import ts
from concourse.bass_utils import run_bass_kernel_spmd

N_CORES = 8
B, T, C, H, D = 4, 2048, 1024, 16, 64
CB = C // 128          # 8 contraction blocks
NKB = T // 128         # 16 key blocks
NQC = T // 512         # 4 query chunks
NEG = -1.0e9

F32 = mybir.dt.float32
BF16 = mybir.dt.bfloat16
AF = mybir.ActivationFunctionType
OP = mybir.AluOpType

_CACHE = {}


def _build(reps=1):
    nc = bacc.Bacc("TRN2", target_bir_lowering=False, debug=False, num_devices=N_CORES)

    xT = nc.dram_tensor("xT", [C, T], BF16, kind="ExternalInput").ap()
    w_qk = nc.dram_tensor("w_qk", [C, 1024], BF16, kind="ExternalInput").ap()
    w_v = nc.dram_tensor("w_v", [C, 512], BF16, kind="ExternalInput").ap()
    w_pr = nc.dram_tensor("w_pr", [512, C], BF16, kind="ExternalInput").ap()
    b_qk = nc.dram_tensor("b_qk", [1024], F32, kind="ExternalInput").ap()
    b_v = nc.dram_tensor("b_v", [128, 8, 64], F32, kind="ExternalInput").ap()
    yT = nc.dram_tensor("yT", [C, T], BF16, kind="ExternalOutput").ap()

    xT_r = xT.rearrange("(cb p) t -> p cb t", p=128)
    w_qk_r = w_qk.rearrange("(cb p) m -> p cb m", p=128)
    w_v_r = w_v.rearrange("(cb p) m -> p cb m", p=128)
    w_pr_r = w_pr.rearrange("(pb p) m -> p pb m", p=128)
    b_qk_r = b_qk.rearrange("(m p) -> p m", p=128)
    yT_r = yT.rearrange("(m p) t -> p m t", p=128)

    with tile.TileContext(nc) as tc:
        with ExitStack() as ctx:
            io = ctx.enter_context(tc.tile_pool(name="io", bufs=2))
            wqk_p = ctx.enter_context(tc.tile_pool(name="wqk", bufs=1))
            w2_p = ctx.enter_context(tc.tile_pool(name="w2", bufs=1))
            wpr_p = ctx.enter_context(tc.tile_pool(name="wpr", bufs=1))
            # per-rep rotation (bufs=2) so next-rep QKV filler writes never
            # WAR against this rep's pending attention reads (DVE-queue cycle)
            qkt_p = ctx.enter_context(tc.tile_pool(name="qkt", bufs=2))
            vaug_p = ctx.enter_context(tc.tile_pool(name="vaug", bufs=2))
            p_p = ctx.enter_context(tc.tile_pool(name="pp", bufs=4))
            ot_p = ctx.enter_context(tc.tile_pool(name="ot", bufs=1))
            misc = ctx.enter_context(tc.tile_pool(name="misc", bufs=1))
            rec_p = ctx.enter_context(tc.tile_pool(name="rec", bufs=2))
            stage_p = ctx.enter_context(tc.tile_pool(name="stage", bufs=3))
            ps_s_p = ctx.enter_context(tc.tile_pool(name="ps_s", bufs=4, space="PSUM"))
            ps_o_p = ctx.enter_context(tc.tile_pool(name="ps_o", bufs=2, space="PSUM"))
            ps_f_p = ctx.enter_context(tc.tile_pool(name="ps_f", bufs=2, space="PSUM"))

            # constants
            b_qk_sb = misc.tile([128, 8], F32)
            nc.sync.dma_start(b_qk_sb[:], b_qk_r)
            b_v_sb = misc.tile([128, 8, 64], F32)
            nc.sync.dma_start(b_v_sb[:], b_v)
            ones_sb = misc.tile([128, 64], F32)
            nc.gpsimd.memset(ones_sb[:], 1.0)
            tri = misc.tile([128, 128], F32)
            nc.gpsimd.memset(tri[:], 0.0)
            # 0 where q(free) >= k(partition), NEG where q < k
            nc.gpsimd.affine_select(
                out=tri[:], in_=tri[:], compare_op=OP.is_ge, fill=NEG,
                base=0, pattern=[[1, 128]], channel_multiplier=-1,
            )

            # first x chunk before the weights: it gates the first matmul
            x_t0 = io.tile([128, CB, 512], BF16, tag="io", name="x_0_0")
            for cb in range(CB):
                nc.sync.dma_start(x_t0[:, cb], xT_r[:, cb, ts(0, 512)])
            # weights (w_qk split per cb so the first matmuls gate on 1/8 of it)
            w_qk_sb = wqk_p.tile([128, CB, 1024], BF16)
            for cb in range(CB):
                nc.sync.dma_start(w_qk_sb[:, cb], w_qk_r[:, cb])
            w_v_sb = w2_p.tile([128, CB, 512], BF16)
            nc.sync.dma_start(w_v_sb[:], w_v_r)
            w_pr_sb = wpr_p.tile([128, 4, 1024], BF16)
            nc.sync.dma_start(w_pr_sb[:], w_pr_r)

            # per-rep SBUF state (rotated): cur["qkT"][:, m, t]: m=0..3 q
            # head-pairs, m=4..7 k head-pairs; cur["vaug"][:, kb, hl, 0:64]=v
            # head hl, [64:128]=ones (sums trick)
            cur = {}
            oT = ot_p.tile([128, 4, T], BF16, name="oT")

            def alloc_rep_state(rep):
                cur["qkT"] = qkt_p.tile([128, 8, T], BF16, tag="qkT",
                                        name=f"qkT_sb_{rep}")
                cur["vaug"] = vaug_p.tile([128, NKB, 8, 128], BF16, tag="vaug",
                                          name=f"v_aug_{rep}")
                nc.vector.tensor_copy(
                    cur["vaug"][:, :, :, 64:128],
                    ones_sb[:, None, None, :].to_broadcast((128, NKB, 8, 64)),
                )

            def qkv_gen(tch, rep, x_pre=None):
                """QKV for token chunk tch; one yield ~= 2 matmuls of filler."""
                if tch == 0:
                    alloc_rep_state(rep)
                qkT_sb, v_aug = cur["qkT"], cur["vaug"]
                if x_pre is not None:
                    x_t = x_pre
                else:
                    x_t = io.tile([128, CB, 512], BF16, tag="io",
                                  name=f"x_{rep}_{tch}")
                    for cb in range(CB):
                        nc.sync.dma_start(x_t[:, cb], xT_r[:, cb, ts(tch, 512)])
                yield
                for m in range(8):
                    ps = ps_f_p.tile([128, 512], F32, tag="f",
                                     name=f"qk_{rep}_{tch}_{m}")
                    for cb in range(CB):
                        nc.tensor.matmul(
                            ps[:], w_qk_sb[:, cb, ts(m, 128)], x_t[:, cb],
                            start=(cb == 0), stop=(cb == CB - 1),
                        )
                        if cb % 2 == 1:
                            yield
                    nc.vector.tensor_tensor(
                        qkT_sb[:, m, ts(tch, 512)], ps[:],
                        b_qk_sb[:, m : m + 1].to_broadcast((128, 512)),
                        OP.add,
                    )
                    yield
                for tq in range(4):
                    tb = tch * 4 + tq   # key block index 0..15
                    ps = ps_f_p.tile([128, 512], F32, tag="f",
                                     name=f"v_{rep}_{tb}")
                    for cb in range(CB):
                        nc.tensor.matmul(
                            ps[:], x_t[:, cb, ts(tq, 128)], w_v_sb[:, cb],
                            start=(cb == 0), stop=(cb == CB - 1),
                        )
                        if cb % 2 == 1:
                            yield
                    nc.vector.tensor_tensor(
                        v_aug[:, tb, :, 0:64],
                        ps[:].rearrange("p (g d) -> p g d", d=64),
                        b_v_sb[:], OP.add,
                    )
                    yield

            def proj_gen(tch, rep):
                """Projection for token chunk tch (needs oT[:, :, tch] done)."""
                for m in range(8):
                    ps = ps_f_p.tile([128, 512], F32, tag="f",
                                     name=f"y_{rep}_{m}_{tch}")
                    for pb in range(4):
                        nc.tensor.matmul(
                            ps[:], w_pr_sb[:, pb, ts(m, 128)],
                            oT[:, pb, ts(tch, 512)],
                            start=(pb == 0), stop=(pb == 3),
                        )
                        if pb % 2 == 1:
                            yield
                    st = stage_p.tile([128, 512], BF16, tag="stage",
                                      name=f"ys_{rep}_{m}_{tch}")
                    nc.vector.tensor_copy(st[:], ps[:])
                    nc.sync.dma_start(yT_r[:, m, ts(tch, 512)], st[:])
                    yield

            def window(qc, rep, fillers, n_units):
                """Attention for query chunk qc, single-head bursts, 3-deep
                S pipeline, paced round-robin filler emission."""
                qkT_sb, v_aug = cur["qkT"], cur["vaug"]
                nkb = 4 * qc + 4
                steps_total = 8 * nkb
                live = list(fillers)
                fstate = {"rr": 0, "left": n_units, "steps": steps_total}

                def advance(n):
                    k = 0
                    while k < n and live:
                        g = live[fstate["rr"] % len(live)]
                        try:
                            next(g)
                            k += 1
                            fstate["rr"] += 1
                        except StopIteration:
                            live.remove(g)
                    fstate["left"] -= k
                    return k

                def step_quota(kb):
                    # even pacing + warmup at burst start
                    base = fstate["left"] // max(fstate["steps"], 1)
                    extra = 2 if kb == 0 else 0
                    fstate["steps"] -= 1
                    return base + extra

                for pr in range(4):
                    for j in (0, 1):
                        pb = j * 64
                        hl = 2 * pr + j
                        ps_o = ps_o_p.tile([128, 512], F32, tag="ps_o",
                                           name=f"o_{rep}_{pr}_{j}_{qc}")
                        ps_s = [None] * nkb

                        def s_step(kb):
                            r = kb - 4 * qc
                            qlo = 128 * r if r > 0 else 0
                            s = ps_s_p.tile([128, 512], F32, tag="ps_s",
                                            name=f"s_{rep}_{hl}_{qc}_{kb}")
                            nc.tensor.matmul(
                                s[:, qlo:512],
                                qkT_sb[pb : pb + 64, 4 + pr, ts(kb, 128)],
                                qkT_sb[pb : pb + 64, pr,
                                       qc * 512 + qlo : (qc + 1) * 512],
                                start=True, stop=True, tile_position=(pb, 0),
                            )
                            if r >= 0:
                                nc.vector.tensor_tensor(
                                    s[:, qlo : qlo + 128], s[:, qlo : qlo + 128],
                                    tri[:], OP.add,
                                )
                            ps_s[kb] = s

                        def av_step(kb):
                            r = kb - 4 * qc
                            qlo = 128 * r if r > 0 else 0
                            p_t = p_p.tile([128, 512], BF16, tag="p",
                                           name=f"p_{rep}_{hl}_{qc}_{kb}")
                            nc.scalar.activation(
                                p_t[:, qlo:512], ps_s[kb][:, qlo:512],
                                AF.Exp, scale=0.125,
                            )
                            ps_s[kb] = None
                            nc.tensor.matmul(
                                ps_o[:, qlo:512], v_aug[:, kb, hl],
                                p_t[:, qlo:512],
                                start=(kb == 0), stop=(kb == nkb - 1),
                            )

                        for kb0 in range(min(3, nkb)):
                            s_step(kb0)
                        for kb in range(nkb):
                            if kb + 3 < nkb:
                                s_step(kb + 3)
                            advance(step_quota(kb))
                            av_step(kb)

                        rec = rec_p.tile([64, 512], F32, tag="rec",
                                         name=f"rec_{rep}_{hl}_{qc}")
                        nc.vector.reciprocal(rec[:], ps_o[64:128, :])
                        nc.vector.tensor_tensor(
                            oT[pb : pb + 64, pr, ts(qc, 512)],
                            ps_o[0:64, :], rec[:], OP.mult,
                        )
                advance(1 << 30)   # drain leftover fillers at window end

            QKV_UNITS, PROJ_UNITS = 61, 24
            for rep in range(reps):
                for w in range(4):
                    fillers = []
                    n_units = 0
                    if rep == 0 and w == 0:
                        # nothing to overlap yet: emit QKV(0) upfront
                        g = qkv_gen(0, rep, x_pre=x_t0)
                        for _ in g:
                            pass
                    else:
                        fillers.append(proj_gen((w - 1) % 4, rep if w else rep - 1))
                        n_units += PROJ_UNITS
                    if w < 3:
                        fillers.append(qkv_gen(w + 1, rep))
                        n_units += QKV_UNITS
                    elif rep + 1 < reps:
                        fillers.append(qkv_gen(0, rep + 1))
                        n_units += QKV_UNITS
                    window(w, rep, fillers, n_units)
                # final rep: tail projection for the last chunk
                if rep == reps - 1:
                    g = proj_gen(3, rep)
                    for _ in g:
                        pass

    nc.compile()
    return nc


def _in_maps(x, W_attn, b_attn, W_proj, b_proj):
    import ml_dtypes
    bf16 = ml_dtypes.bfloat16
    maps = []
    for b in range(B):
        xTb = np.ascontiguousarray(x[b].T.astype(bf16))
        for g in range(2):
            cs = slice(g * 512, (g + 1) * 512)
            maps.append({
                "xT": xTb,
                "w_qk": np.ascontiguousarray(np.concatenate(
                    [W_attn[:, cs], W_attn[:, 1024 + cs.start : 1024 + cs.stop]],
                    axis=1).astype(bf16)),
                "w_v": np.ascontiguousarray(
                    W_attn[:, 2048 + cs.start : 2048 + cs.stop].astype(bf16)),
                "w_pr": np.ascontiguousarray(W_proj[cs, :].astype(bf16)),
                "b_qk": np.ascontiguousarray(np.concatenate(
                    [b_attn[cs], b_attn[1024 + cs.start : 1024 + cs.stop]])),
                "b_v": np.ascontiguousarray(np.tile(
                    b_attn[2048 + cs.start : 2048 + cs.stop][None, :],
                    (128, 1)).reshape(128, 8, 64)),
            })
    return maps


def kernel(x, W_attn, b_attn, W_proj, b_proj):
    x = np.asarray(x, dtype=np.float32)
    W_attn = np.asarray(W_attn, dtype=np.float32)
    b_attn = np.asarray(b_attn, dtype=np.float32)
    W_proj = np.asarray(W_proj, dtype=np.float32)
    b_proj = np.asarray(b_proj, dtype=np.float32)

    if "nc" not in _CACHE:
        _CACHE["nc"] = _build()
    nc = _CACHE["nc"]

    maps = _in_maps(x, W_attn, b_attn, W_proj, b_proj)
    last_exc = None
    for attempt in range(3):
        try:
            res = run_bass_kernel_spmd(nc, maps, core_ids=list(range(N_CORES)))
            break
        except Exception as exc:  # transient device wedges recover on retry
            last_exc = exc
            if attempt == 2:
                raise
            import time as _time
            _time.sleep(5)
    y = np.empty((B, T, C), dtype=np.float32)
    for b in range(B):
        y[b] = (res.results[2 * b]["yT"].astype(np.float32)
                + res.results[2 * b + 1]["yT"].astype(np.float32)).T + b_proj
    return y


# revision 3
# speedup vs baseline: 1.8807x; 1.8807x over previous
"""Causal self-attention TRN2 Bass kernel (B=4, T=2048, C=1024, H=16, D=64).

Sharding: 8 cores = 4 batches x 2 head-groups (8 heads each); host sums the
two head-group partial projections per batch and adds b_proj.

All matmuls bf16 (tolerance 2e-2); q/k/v SBUF-resident (no DRAM round trips);
v written at QKV time directly into the ones-augmented AV layout (out rows
0..63 = o^T, 64..127 = softmax sums).

Scheduling: softmax exp on ACT (~612ns/step) exceeds the per-head S+AV PE
work per attention step (~426ns), so attention alone is ACT-latency-bound.
This kernel makes PE work-conserving: QKV for token chunk tch+1 and the
projection for chunk tch-1 are emitted as paced filler between attention
steps of chunk tch (causality: query chunk tch needs K/V only through chunk
tch). ACT does exp ONLY; all bias/copy epilogues run on DVE. Attention runs
single-head bursts with a 3-deep S pipeline so exp latency is always covered.
PSUM: 4 S-tiles (4 banks) + 2 O-tiles (2) + 2 filler accumulators (2) = 8.
qkT/v_aug rotate per rep (bufs=2) so next-rep filler writes never WAR against
this rep's pending attention reads (would cycle through the in-order DVE
queue with the PE queue = deadlock).
"""

import numpy as np
from contextlib import ExitStack

import concourse.bass as bass
import concourse.tile as tile
from concourse import bacc, mybir
from concourse.bass import ts
from concourse.bass_utils import run_bass_kernel_spmd

N_CORES = 8
B, T, C, H, D = 4, 2048, 1024, 16, 64
CB = C // 128          # 8 contraction blocks
NKB = T // 128         # 16 key blocks
NQC = T // 512         # 4 query chunks
NEG = -1.0e9

F32 = mybir.dt.float32
BF16 = mybir.dt.bfloat16
AF = mybir.ActivationFunctionType
OP = mybir.AluOpType

_CACHE = {}


def _build(reps=1):
    nc = bacc.Bacc("TRN2", target_bir_lowering=False, debug=False, num_devices=N_CORES)

    xT = nc.dram_tensor("xT", [C, T], BF16, kind="ExternalInput").ap()
    w_qk = nc.dram_tensor("w_qk", [C, 1024], BF16, kind="ExternalInput").ap()
    w_v = nc.dram_tensor("w_v", [C, 512], BF16, kind="ExternalInput").ap()
    w_pr = nc.dram_tensor("w_pr", [512, C], BF16, kind="ExternalInput").ap()
    b_qk = nc.dram_tensor("b_qk", [1024], F32, kind="ExternalInput").ap()
    b_v = nc.dram_tensor("b_v", [128, 8, 64], F32, kind="ExternalInput").ap()
    yT = nc.dram_tensor("yT", [C, T], BF16, kind="ExternalOutput").ap()

    xT_r = xT.rearrange("(cb p) t -> p cb t", p=128)
    w_qk_r = w_qk.rearrange("(cb p) m -> p cb m", p=128)
    w_v_r = w_v.rearrange("(cb p) m -> p cb m", p=128)
    w_pr_r = w_pr.rearrange("(pb p) m -> p pb m", p=128)
    b_qk_r = b_qk.rearrange("(m p) -> p m", p=128)
    yT_r = yT.rearrange("(m p) t -> p m t", p=128)

    with tile.TileContext(nc) as tc:
        with ExitStack() as ctx:
            io = ctx.enter_context(tc.tile_pool(name="io", bufs=2))
            wqk_p = ctx.enter_context(tc.tile_pool(name="wqk", bufs=1))
            w2_p = ctx.enter_context(tc.tile_pool(name="w2", bufs=1))
            wpr_p = ctx.enter_context(tc.tile_pool(name="wpr", bufs=1))
            # per-rep rotation (bufs=2) so next-rep QKV filler writes never
            # WAR against this rep's pending attention reads (DVE-queue cycle)
            qkt_p = ctx.enter_context(tc.tile_pool(name="qkt", bufs=2))
            vaug_p = ctx.enter_context(tc.tile_pool(name="vaug", bufs=2))
            p_p = ctx.enter_context(tc.tile_pool(name="pp", bufs=4))
            ot_p = ctx.enter_context(tc.tile_pool(name="ot", bufs=1))
            misc = ctx.enter_context(tc.tile_pool(name="misc", bufs=1))
            rec_p = ctx.enter_context(tc.tile_pool(name="rec", bufs=2))
            stage_p = ctx.enter_context(tc.tile_pool(name="stage", bufs=3))
            ps_s_p = ctx.enter_context(tc.tile_pool(name="ps_s", bufs=4, space="PSUM"))
            ps_o_p = ctx.enter_context(tc.tile_pool(name="ps_o", bufs=2, space="PSUM"))
            ps_f_p = ctx.enter_context(tc.tile_pool(name="ps_f", bufs=2, space="PSUM"))

            # constants
            b_qk_sb = misc.tile([128, 8], F32)
            nc.sync.dma_start(b_qk_sb[:], b_qk_r)
            b_v_sb = misc.tile([128, 8, 64], F32)
            nc.sync.dma_start(b_v_sb[:], b_v)
            ones_sb = misc.tile([128, 64], F32)
            nc.gpsimd.memset(ones_sb[:], 1.0)
            tri = misc.tile([128, 128], F32)
            nc.gpsimd.memset(tri[:], 0.0)
            # 0 where q(free) >= k(partition), NEG where q < k
            nc.gpsimd.affine_select(
                out=tri[:], in_=tri[:], compare_op=OP.is_ge, fill=NEG,
                base=0, pattern=[[1, 128]], channel_multiplier=-1,
            )

            # first x chunk before the weights: it gates the first matmul
            x_t0 = io.tile([128, CB, 512], BF16, tag="io", name="x_0_0")
            for cb in range(CB):
                nc.sync.dma_start(x_t0[:, cb], xT_r[:, cb, ts(0, 512)])
            # weights (w_qk split per cb so the first matmuls gate on 1/8 of it)
            w_qk_sb = wqk_p.tile([128, CB, 1024], BF16)
            for cb in range(CB):
                nc.sync.dma_start(w_qk_sb[:, cb], w_qk_r[:, cb])
            w_v_sb = w2_p.tile([128, CB, 512], BF16)
            nc.sync.dma_start(w_v_sb[:], w_v_r)
            w_pr_sb = wpr_p.tile([128, 4, 1024], BF16)
            nc.sync.dma_start(w_pr_sb[:], w_pr_r)

            # per-rep SBUF state (rotated): cur["qkT"][:, m, t]: m=0..3 q
            # head-pairs, m=4..7 k head-pairs; cur["vaug"][:, kb, hl, 0:64]=v
            # head hl, [64:128]=ones (sums trick)
            cur = {}
            oT = ot_p.tile([128, 4, T], BF16, name="oT")

            def alloc_rep_state(rep):
                cur["qkT"] = qkt_p.tile([128, 8, T], BF16, tag="qkT",
                                        name=f"qkT_sb_{rep}")
                cur["vaug"] = vaug_p.tile([128, NKB, 8, 128], BF16, tag="vaug",
                                          name=f"v_aug_{rep}")
                nc.vector.tensor_copy(
                    cur["vaug"][:, :, :, 64:128],
                    ones_sb[:, None, None, :].to_broadcast((128, NKB, 8, 64)),
                )

            def qkv_gen(tch, rep, x_pre=None):
                """QKV for token chunk tch; one yield ~= 2 matmuls of filler."""
                if tch == 0:
                    alloc_rep_state(rep)
                qkT_sb, v_aug = cur["qkT"], cur["vaug"]
                if x_pre is not None:
                    x_t = x_pre
                else:
                    x_t = io.tile([128, CB, 512], BF16, tag="io",
                                  name=f"x_{rep}_{tch}")
                    for cb in range(CB):
                        nc.sync.dma_start(x_t[:, cb], xT_r[:, cb, ts(tch, 512)])
                yield
                for m in range(8):
                    ps = ps_f_p.tile([128, 512], F32, tag="f",
                                     name=f"qk_{rep}_{tch}_{m}")
                    for cb in range(CB):
                        nc.tensor.matmul(
                            ps[:], w_qk_sb[:, cb, ts(m, 128)], x_t[:, cb],
                            start=(cb == 0), stop=(cb == CB - 1),
                        )
                        if cb % 2 == 1:
                            yield
                    nc.vector.tensor_tensor(
                        qkT_sb[:, m, ts(tch, 512)], ps[:],
                        b_qk_sb[:, m : m + 1].to_broadcast((128, 512)),
                        OP.add,
                    )
                    yield
                for tq in range(4):
                    tb = tch * 4 + tq   # key block index 0..15
                    ps = ps_f_p.tile([128, 512], F32, tag="f",
                                     name=f"v_{rep}_{tb}")
                    for cb in range(CB):
                        nc.tensor.matmul(
                            ps[:], x_t[:, cb, ts(tq, 128)], w_v_sb[:, cb],
                            start=(cb == 0), stop=(cb == CB - 1),
                        )
                        if cb % 2 == 1:
                            yield
                    nc.vector.tensor_tensor(
                        v_aug[:, tb, :, 0:64],
                        ps[:].rearrange("p (g d) -> p g d", d=64),
                        b_v_sb[:], OP.add,
                    )
                    yield

            def proj_gen(tch, rep):
                """Projection for token chunk tch (needs oT[:, :, tch] done)."""
                for m in range(8):
                    ps = ps_f_p.tile([128, 512], F32, tag="f",
                                     name=f"y_{rep}_{m}_{tch}")
                    for pb in range(4):
                        nc.tensor.matmul(
                            ps[:], w_pr_sb[:, pb, ts(m, 128)],
                            oT[:, pb, ts(tch, 512)],
                            start=(pb == 0), stop=(pb == 3),
                        )
                        if pb % 2 == 1:
                            yield
                    st = stage_p.tile([128, 512], BF16, tag="stage",
                                      name=f"ys_{rep}_{m}_{tch}")
                    nc.vector.tensor_copy(st[:], ps[:])
                    nc.sync.dma_start(yT_r[:, m, ts(tch, 512)], st[:])
                    yield

            def window(qc, rep, fillers, n_units):
                """Attention for query chunk qc, single-head bursts, 3-deep
                S pipeline, paced round-robin filler emission."""
                qkT_sb, v_aug = cur["qkT"], cur["vaug"]
                nkb = 4 * qc + 4
                steps_total = 8 * nkb
                live = list(fillers)
                fstate = {"rr": 0, "left": n_units, "steps": steps_total}

                def advance(n):
                    k = 0
                    while k < n and live:
                        g = live[fstate["rr"] % len(live)]
                        try:
                            next(g)
                            k += 1
                            fstate["rr"] += 1
                        except StopIteration:
                            live.remove(g)
                    fstate["left"] -= k
                    return k

                def step_quota(kb):
                    # even pacing + warmup at burst start
                    base = fstate["left"] // max(fstate["steps"], 1)
                    extra = 2 if kb == 0 else 0
                    fstate["steps"] -= 1
                    return base + extra

                for pr in range(4):
                    for j in (0, 1):
                        pb = j * 64
                        hl = 2 * pr + j
                        ps_o = ps_o_p.tile([128, 512], F32, tag="ps_o",
                                           name=f"o_{rep}_{pr}_{j}_{qc}")
                        ps_s = [None] * nkb

                        def s_step(kb):
                            r = kb - 4 * qc
                            qlo = 128 * r if r > 0 else 0
                            s = ps_s_p.tile([128, 512], F32, tag="ps_s",
                                            name=f"s_{rep}_{hl}_{qc}_{kb}")
                            nc.tensor.matmul(
                                s[:, qlo:512],
                                qkT_sb[pb : pb + 64, 4 + pr, ts(kb, 128)],
                                qkT_sb[pb : pb + 64, pr,
                                       qc * 512 + qlo : (qc + 1) * 512],
                                start=True, stop=True, tile_position=(pb, 0),
                            )
                            if r >= 0:
                                nc.vector.tensor_tensor(
                                    s[:, qlo : qlo + 128], s[:, qlo : qlo + 128],
                                    tri[:], OP.add,
                                )
                            ps_s[kb] = s

                        def av_step(kb):
                            r = kb - 4 * qc
                            qlo = 128 * r if r > 0 else 0
                            p_t = p_p.tile([128, 512], BF16, tag="p",
                                           name=f"p_{rep}_{hl}_{qc}_{kb}")
                            nc.scalar.activation(
                                p_t[:, qlo:512], ps_s[kb][:, qlo:512],
                                AF.Exp, scale=0.125,
                            )
                            ps_s[kb] = None
                            nc.tensor.matmul(
                                ps_o[:, qlo:512], v_aug[:, kb, hl],
                                p_t[:, qlo:512],
                                start=(kb == 0), stop=(kb == nkb - 1),
                            )

                        for kb0 in range(min(3, nkb)):
                            s_step(kb0)
                        for kb in range(nkb):
                            if kb + 3 < nkb:
                                s_step(kb + 3)
                            advance(step_quota(kb))
                            av_step(kb)

                        rec = rec_p.tile([64, 512], F32, tag="rec",
                                         name=f"rec_{rep}_{hl}_{qc}")
                        nc.vector.reciprocal(rec[:], ps_o[64:128, :])
                        nc.vector.tensor_tensor(
                            oT[pb : pb + 64, pr, ts(qc, 512)],
                            ps_o[0:64, :], rec[:], OP.mult,
                        )
                advance(1 << 30)   # drain leftover fillers at window end

            QKV_UNITS, PROJ_UNITS = 61, 24
            for rep in range(reps):
                for w in range(4):
                    fillers = []
                    n_units = 0
                    if rep == 0 and w == 0:
                        # nothing to overlap yet: emit QKV(0) upfront
                        g = qkv_gen(0, rep, x_pre=x_t0)
                        for _ in g:
                            pass
                    else:
                        fillers.append(proj_gen((w - 1) % 4, rep if w else rep - 1))
                        n_units += PROJ_UNITS
                    if w < 3:
                        fillers.append(qkv_gen(w + 1, rep))
                        n_units += QKV_UNITS
                    elif rep + 1 < reps:
                        fillers.append(qkv_gen(0, rep + 1))
                        n_units += QKV_UNITS
                    window(w, rep, fillers, n_units)
                # final rep: tail projection for the last chunk
                if rep == reps - 1:
                    g = proj_gen(3, rep)
                    for _ in g:
                        pass

    nc.compile()
    return nc


def _in_maps(x, W_attn, b_attn, W_proj, b_proj):
    import ml_dtypes
    bf16 = ml_dtypes.bfloat16
    maps = []
    for b in range(B):
        xTb = np.ascontiguousarray(x[b].T.astype(bf16))
        for g in range(2):
            cs = slice(g * 512, (g + 1) * 512)
            maps.append({
                "xT": xTb,
                "w_qk": np.ascontiguousarray(np.concatenate(
                    [W_attn[:, cs], W_attn[:, 1024 + cs.start : 1024 + cs.stop]],
                    axis=1).astype(bf16)),
                "w_v": np.ascontiguousarray(
                    W_attn[:, 2048 + cs.start : 2048 + cs.stop].astype(bf16)),
                "w_pr": np.ascontiguousarray(W_proj[cs, :].astype(bf16)),
                "b_qk": np.ascontiguousarray(np.concatenate(
                    [b_attn[cs], b_attn[1024 + cs.start : 1024 + cs.stop]])),
                "b_v": np.ascontiguousarray(np.tile(
                    b_attn[2048 + cs.start : 2048 + cs.stop][None, :],
                    (128, 1)).reshape(128, 8, 64)),
            })
    return maps


def kernel(x, W_attn, b_attn, W_proj, b_proj):
    x = np.asarray(x, dtype=np.float32)
    W_attn = np.asarray(W_attn, dtype=np.float32)
    b_attn = np.asarray(b_attn, dtype=np.float32)
    W_proj = np.asarray(W_proj, dtype=np.float32)
    b_proj = np.asarray(b_proj, dtype=np.float32)

    if "nc" not in _CACHE:
        _CACHE["nc"] = _build()
    nc = _CACHE["nc"]

    maps = _in_maps(x, W_attn, b_attn, W_proj, b_proj)
    last_exc = None
    for attempt in range(3):
        try:
            res = run_bass_kernel_spmd(nc, maps, core_ids=list(range(N_CORES)))
            break
        except Exception as exc:  # transient device wedges recover on retry
            last_exc = exc
            if attempt == 2:
                raise
            import time as _time
            _time.sleep(5)
    y = np.empty((B, T, C), dtype=np.float32)
    for b in range(B):
        y[b] = (res.results[2 * b]["yT"].astype(np.float32)
                + res.results[2 * b + 1]["yT"].astype(np.float32)).T + b_proj
    return y
